# revision 18
# baseline (speedup 1.0000x reference)
"""AlternateTimelineGenerator Trainium2 kernel.

Data-parallel over the batch: 16384 rows -> 8 NeuronCores x 2048 rows.
Per core, everything runs in "transposed" layout (feature on SBUF
partitions, batch rows on the free dimension) so the whole chain of
matmuls composes without on-chip transposes; only the two inputs are
transposed once at the start via TensorE. Outputs are written transposed
(fp16 for the big timeline tensor) and de-transposed/upcast on the host.

Precision plan (validated in numpy against an fp64 oracle):
  fp32: PSUM accumulation, the P = W_pe1@cur + b state, s output
  fp16: weights, sigmoid/tanh outputs, mod, cur, c0/c1
  fp8 (e4m3, DoubleRow matmuls at 0.5 cyc/row): h0, h1, rom

Per step (full 2048-row spans, matmul N=512 slices):
  gates0 = DR(W_hh0 @ h0) + W_ih0 @ s          (PE, PSUM fp32)
  i,f,g,o = ACT sigmoid/tanh (+bias, FD=2048)  -> fp16 SBUF
  c = f*c + i*g; h = o*tanh(c)                 (DVE fp16, h cast to fp8)
  gates1 = DR(W_ih1 @ h0) + DR(W_hh1 @ h1)
  rom  = relu(DR(W_om1h @ h1) + W_om1s @ s)    (PE + DVE relu -> fp8)
  mod  = tanh(DR(W_om2 @ rom) + b)             (PE + ACT -> fp16)
  P   += (0.1*W_pe1) @ mod                     (PE fp16; P = W_pe1@cur + b)
  logit= W_pe2 @ relu(P)                       (PE fp16; sigmoid on host)
  cur += 0.1 * mod                             (DVE fp16)
"""

import numpy as np
import ml_dtypes

B, FDIM, H, E = 16384, 512, 256, 128
NCORES = 8
R = B // NCORES        # rows per core
P = 128
MM = 512               # matmul moving-dim tile

_BUILD_CACHE = {}


def _build(num_steps: int, R: int = R):
    import concourse.bass as bass
    import concourse.tile as tile
    from concourse import bacc, mybir
    from concourse.masks import make_identity

    NSL = R // MM      # matmul N-slices per full span
    dt = mybir.dt
    AF = mybir.ActivationFunctionType
    ALU = mybir.AluOpType
    DR = mybir.MatmulPerfMode.DoubleRow
    f32, f16, f8 = dt.float32, dt.float16, dt.float8e4

    nc = bacc.Bacc("TRN2", target_bir_lowering=False, debug=False)

    # ---- DRAM I/O ----
    base_d = nc.dram_tensor("base", [R, FDIM], f32, kind="ExternalInput").ap()
    cf_d = nc.dram_tensor("cf", [R, FDIM], f32, kind="ExternalInput").ap()
    wt_g0h_d = nc.dram_tensor("wt_g0h", [2 * P, 4 * H], f8, kind="ExternalInput").ap()
    wt_g0s_d = nc.dram_tensor("wt_g0s", [P, 4 * H], f16, kind="ExternalInput").ap()
    wt_g1_d = nc.dram_tensor("wt_g1", [4 * P, 4 * H], f8, kind="ExternalInput").ap()
    wt_om1h_d = nc.dram_tensor("wt_om1h", [2 * P, 256], f8, kind="ExternalInput").ap()
    wt_om1s_d = nc.dram_tensor("wt_om1s", [P, 256], f16, kind="ExternalInput").ap()
    wt_om2_d = nc.dram_tensor("wt_om2", [2 * P, FDIM], f8, kind="ExternalInput").ap()
    wt_se1_d = nc.dram_tensor("wt_se1", [FDIM, 256], f16, kind="ExternalInput").ap()
    wt_se2_d = nc.dram_tensor("wt_se2", [256, E], f16, kind="ExternalInput").ap()
    wt_pe1_d = nc.dram_tensor("wt_pe1", [FDIM, E], f16, kind="ExternalInput").ap()
    wt_pe1s_d = nc.dram_tensor("wt_pe1s", [FDIM, E], f16, kind="ExternalInput").ap()
    wt_pe2_d = nc.dram_tensor("wt_pe2", [E, 1], f16, kind="ExternalInput").ap()
    bg0_d = nc.dram_tensor("bg0", [4 * H], f32, kind="ExternalInput").ap()
    bg1_d = nc.dram_tensor("bg1", [4 * H], f32, kind="ExternalInput").ap()
    bse1_d = nc.dram_tensor("b_se1", [256], f32, kind="ExternalInput").ap()
    bse2_d = nc.dram_tensor("b_se2", [E], f32, kind="ExternalInput").ap()
    bom1_d = nc.dram_tensor("b_om1", [256], f32, kind="ExternalInput").ap()
    bom2_d = nc.dram_tensor("b_om2", [FDIM], f32, kind="ExternalInput").ap()
    bpe1_d = nc.dram_tensor("b_pe1", [E], f32, kind="ExternalInput").ap()

    alt_d = nc.dram_tensor("alt_t", [num_steps, FDIM, R], f16, kind="ExternalOutput").ap()
    probs_d = nc.dram_tensor("probs_t", [num_steps, R], f16, kind="ExternalOutput").ap()
    s_d = nc.dram_tensor("s_t", [E, R], f32, kind="ExternalOutput").ap()

    with tile.TileContext(nc) as tc:
        with (
            tc.tile_pool(name="consts", bufs=1) as consts,
            tc.tile_pool(name="state", bufs=1) as state,
            tc.tile_pool(name="psum", bufs=2, space="PSUM") as psum,
        ):
            # ---- weights / biases to SBUF ----
            def w_tile(name, dram, kk, mm_, dtype):
                tl = consts.tile([P, kk, mm_], dtype, tag=name)
                nc.sync.dma_start(tl[:], dram.rearrange("(k p) m -> p k m", p=P))
                return tl

            wt_g0h = w_tile("wt_g0h", wt_g0h_d, 2, 4 * H, f8)
            wt_g0s = w_tile("wt_g0s", wt_g0s_d, 1, 4 * H, f16)
            wt_g1 = w_tile("wt_g1", wt_g1_d, 4, 4 * H, f8)
            wt_om1h = w_tile("wt_om1h", wt_om1h_d, 2, 256, f8)
            wt_om1s = w_tile("wt_om1s", wt_om1s_d, 1, 256, f16)
            wt_om2 = w_tile("wt_om2", wt_om2_d, 2, FDIM, f8)
            wt_se1 = w_tile("wt_se1", wt_se1_d, 4, 256, f16)
            wt_se2 = w_tile("wt_se2", wt_se2_d, 2, E, f16)
            wt_pe1 = w_tile("wt_pe1", wt_pe1_d, 4, E, f16)
            wt_pe1s = w_tile("wt_pe1s", wt_pe1s_d, 4, E, f16)
            wt_pe2 = consts.tile([P, 1], f16, tag="wt_pe2")
            nc.sync.dma_start(wt_pe2[:], wt_pe2_d)

            def bias_tile(name, dram, m):
                tl = consts.tile([P, m], f32, tag=name)
                nc.sync.dma_start(tl[:], dram.rearrange("(m p) -> p m", p=P))
                return tl

            bg0 = bias_tile("bg0", bg0_d, 8)
            bg1 = bias_tile("bg1", bg1_d, 8)
            bse1 = bias_tile("bse1", bse1_d, 2)
            bse2 = bias_tile("bse2", bse2_d, 1)
            bom1 = bias_tile("bom1", bom1_d, 2)
            bom2 = bias_tile("bom2", bom2_d, 4)
            bpe1 = bias_tile("bpe1", bpe1_d, 1)

            ident = consts.tile([P, P], f32, tag="ident")
            make_identity(nc, ident[:])

            # ---- persistent state ----
            h0 = state.tile([P, 2, R], f8, tag="h0")
            h1 = state.tile([P, 2, R], f8, tag="h1")
            c0 = state.tile([P, 2, R], f16, tag="c0")
            c1 = state.tile([P, 2, R], f16, tag="c1")
            cur = state.tile([P, 4, R], f16, tag="cur")
            pacc = state.tile([P, R], f32, tag="pacc")
            s = state.tile([P, R], f32, tag="s")
            sbf = state.tile([P, R], f16, tag="sbf")
            for tl in (h0, h1, c0, c1):
                nc.vector.memset(tl[:], 0.0)

            # ---- transpose inputs (cf -> cfT f16, base -> cur f16) ----
            with tc.tile_pool(name="setup", bufs=3) as setup, \
                 tc.tile_pool(name="setup1", bufs=1) as setup1:
                cfT = setup1.tile([P, 4, R], f16, tag="cfT")
                r1 = setup1.tile([P, 2, R], f16, tag="r1")
                for src, dst in ((cf_d, cfT), (base_d, cur)):
                    for rt in range(R // P):
                        tmp = setup.tile([P, FDIM], f32, tag="tr_in")
                        nc.sync.dma_start(tmp[:], src[rt * P:(rt + 1) * P, :])
                        pt = psum.tile([P, 4, P], f32, tag="ps")
                        for ft in range(4):
                            nc.tensor.transpose(
                                pt[:, ft, :], tmp[:, ft * P:(ft + 1) * P], ident[:]
                            )
                        nc.vector.tensor_copy(dst[:, :, rt * P:(rt + 1) * P], pt[:])

                # ---- scenario encoder (f16) + P0 = W_pe1 @ base^T + b ----
                for n in range(NSL):
                    sl = slice(n * MM, (n + 1) * MM)
                    for m in range(2):
                        ps = psum.tile([P, MM], f32, tag="ps")
                        for k in range(4):
                            nc.tensor.matmul(
                                ps[:], wt_se1[:, k, m * P:(m + 1) * P],
                                cfT[:, k, sl], start=(k == 0), stop=(k == 3),
                            )
                        nc.scalar.activation(
                            r1[:, m, sl], ps[:], AF.Relu, bias=bse1[:, m:m + 1])
                    ps = psum.tile([P, MM], f32, tag="ps")
                    for k in range(2):
                        nc.tensor.matmul(
                            ps[:], wt_se2[:, k, :], r1[:, k, sl],
                            start=(k == 0), stop=(k == 1),
                        )
                    nc.scalar.activation(
                        s[:, sl], ps[:], AF.Identity, bias=bse2[:, 0:1])
                    ps = psum.tile([P, MM], f32, tag="ps")
                    for k in range(4):
                        nc.tensor.matmul(
                            ps[:], wt_pe1[:, k, :], cur[:, k, sl],
                            start=(k == 0), stop=(k == 3),
                        )
                    nc.vector.tensor_scalar(
                        pacc[:, sl], ps[:], bpe1[:, 0:1], None, ALU.add)
                nc.vector.tensor_copy(sbf[:], s[:])
                nc.sync.dma_start(s_d[:, :], s[:])

            # ---- recurrence: full-span phases ----
            with tc.tile_pool(name="loop", bufs=2) as loop:
                for t in range(num_steps):
                    for gate_set in range(2):
                        cc = c0 if gate_set == 0 else c1
                        hh = h0 if gate_set == 0 else h1
                        bgt = bg0 if gate_set == 0 else bg1
                        sg = loop.tile([P, 8, R], f16, tag="sg")
                        for m in range(8):
                            ps = psum.tile([P, R], f32, tag="ps")
                            for n in range(NSL):
                                o = slice(n * MM, (n + 1) * MM)
                                if gate_set == 1:
                                    nc.tensor.matmul(
                                        ps[:, o], wt_g1[:, 0:2, m * P:(m + 1) * P],
                                        h0[:, :, o], start=True, stop=False,
                                        perf_mode=DR)
                                    nc.tensor.matmul(
                                        ps[:, o], wt_g1[:, 2:4, m * P:(m + 1) * P],
                                        h1[:, :, o], start=False, stop=True,
                                        perf_mode=DR)
                                else:
                                    nc.tensor.matmul(
                                        ps[:, o], wt_g0h[:, :, m * P:(m + 1) * P],
                                        h0[:, :, o], start=True, stop=False,
                                        perf_mode=DR)
                                    nc.tensor.matmul(
                                        ps[:, o], wt_g0s[:, 0, m * P:(m + 1) * P],
                                        sbf[:, o], start=False, stop=True)
                            func = AF.Tanh if m in (4, 5) else AF.Sigmoid
                            nc.scalar.activation(
                                sg[:, m, :], ps[:], func, bias=bgt[:, m:m + 1])
                        # LSTM cell elementwise (fp16, full span)
                        nc.vector.tensor_tensor(
                            sg[:, 0:2, :], sg[:, 0:2, :], sg[:, 4:6, :], ALU.mult)
                        nc.vector.tensor_tensor(
                            cc[:], sg[:, 2:4, :], cc[:], ALU.mult)
                        nc.vector.tensor_tensor(
                            cc[:], cc[:], sg[:, 0:2, :], ALU.add)
                        nc.scalar.activation(sg[:, 4:6, :], cc[:], AF.Tanh)
                        nc.vector.tensor_tensor(
                            hh[:], sg[:, 6:8, :], sg[:, 4:6, :], ALU.mult)

                    # outcome modifier
                    rom = loop.tile([P, 2, R], f8, tag="rom")
                    for m in range(2):
                        ps = psum.tile([P, R], f32, tag="ps")
                        for n in range(NSL):
                            o = slice(n * MM, (n + 1) * MM)
                            nc.tensor.matmul(
                                ps[:, o], wt_om1h[:, :, m * P:(m + 1) * P],
                                h1[:, :, o], start=True, stop=False, perf_mode=DR)
                            nc.tensor.matmul(
                                ps[:, o], wt_om1s[:, 0, m * P:(m + 1) * P],
                                sbf[:, o], start=False, stop=True)
                        nc.vector.tensor_scalar(
                            rom[:, m, :], ps[:], bom1[:, m:m + 1], 0.0,
                            ALU.add, ALU.max)
                    mod = loop.tile([P, 4, R], f16, tag="mod", bufs=1)
                    for m in range(4):
                        ps = psum.tile([P, R], f32, tag="ps")
                        for n in range(NSL):
                            o = slice(n * MM, (n + 1) * MM)
                            nc.tensor.matmul(
                                ps[:, o], wt_om2[:, :, m * P:(m + 1) * P],
                                rom[:, :, o], start=True, stop=True, perf_mode=DR)
                        nc.scalar.activation(
                            mod[:, m, :], ps[:], AF.Tanh, bias=bom2[:, m:m + 1])

                    # probability estimator: P += (0.1 W_pe1) @ mod
                    ps = psum.tile([P, R], f32, tag="ps")
                    for n in range(NSL):
                        o = slice(n * MM, (n + 1) * MM)
                        for k in range(4):
                            nc.tensor.matmul(
                                ps[:, o], wt_pe1s[:, k, :], mod[:, k, o],
                                start=(k == 0), stop=(k == 3),
                            )
                    nc.vector.tensor_tensor(pacc[:], pacc[:], ps[:], ALU.add)
                    rp = loop.tile([P, R], f16, tag="rp")
                    nc.vector.tensor_scalar_max(rp[:], pacc[:], 0.0)
                    ps2 = psum.tile([P, R], f32, tag="ps")
                    for n in range(NSL):
                        o = slice(n * MM, (n + 1) * MM)
                        nc.tensor.matmul(
                            ps2[0:1, o], wt_pe2[:, 0:1], rp[:, o],
                            start=True, stop=True,
                        )
                    pb = loop.tile([1, R], f16, tag="pb")
                    nc.vector.tensor_copy(pb[0:1, :], ps2[0:1, :])
                    nc.sync.dma_start(probs_d[t:t + 1, :], pb[0:1, :])

                    # cur += 0.1*mod ; stream out
                    nc.vector.scalar_tensor_tensor(
                        cur[:], mod[:], 0.1, cur[:], ALU.mult, ALU.add)
                    nc.sync.dma_start(
                        alt_d[t].rearrange("(fc p) r -> p fc r", p=P), cur[:])

    nc.compile()
    return nc


def _get_nc(num_steps: int):
    if num_steps not in _BUILD_CACHE:
        _BUILD_CACHE[num_steps] = _build(num_steps)
    return _BUILD_CACHE[num_steps]


def make_in_maps(inputs, R_=None):
    """Shard inputs + preprocess params into per-core input maps."""
    def npf(x):
        return np.asarray(x, np.float32)

    f16 = np.float16
    f8 = ml_dtypes.float8_e4m3fn
    params = {
        "wt_g0h": np.ascontiguousarray(npf(inputs["w_hh0"]).T).astype(f8),
        "wt_g0s": np.ascontiguousarray(npf(inputs["w_ih0"]).T).astype(f16),
        "wt_g1": np.ascontiguousarray(
            np.concatenate([npf(inputs["w_ih1"]).T, npf(inputs["w_hh1"]).T], 0)
        ).astype(f8),
        "wt_om1h": np.ascontiguousarray(npf(inputs["w_om1"]).T[:256]).astype(f8),
        "wt_om1s": np.ascontiguousarray(npf(inputs["w_om1"]).T[256:]).astype(f16),
        "wt_om2": np.ascontiguousarray(npf(inputs["w_om2"]).T).astype(f8),
        "wt_se1": np.ascontiguousarray(npf(inputs["w_se1"]).T).astype(f16),
        "wt_se2": np.ascontiguousarray(npf(inputs["w_se2"]).T).astype(f16),
        "wt_pe1": np.ascontiguousarray(npf(inputs["w_pe1"]).T).astype(f16),
        "wt_pe1s": np.ascontiguousarray(0.1 * npf(inputs["w_pe1"]).T).astype(f16),
        "wt_pe2": np.ascontiguousarray(npf(inputs["w_pe2"]).T).astype(f16),
        "bg0": npf(inputs["b_ih0"]) + npf(inputs["b_hh0"]),
        "bg1": npf(inputs["b_ih1"]) + npf(inputs["b_hh1"]),
        "b_se1": npf(inputs["b_se1"]),
        "b_se2": npf(inputs["b_se2"]),
        "b_om1": npf(inputs["b_om1"]),
        "b_om2": npf(inputs["b_om2"]),
        "b_pe1": npf(inputs["b_pe1"]),
    }
    rr = R_ or R
    base = npf(inputs["base_timeline"])
    cf = npf(inputs["counterfactual_scenario"])
    ncores = base.shape[0] // rr
    in_maps = []
    for i in range(ncores):
        m = dict(params)
        m["base"] = np.ascontiguousarray(base[i * rr:(i + 1) * rr])
        m["cf"] = np.ascontiguousarray(cf[i * rr:(i + 1) * rr])
        in_maps.append(m)
    return in_maps


def assemble(results, b_pe2=0.0):
    """Gather per-core transposed outputs into full reference-shaped arrays."""
    alt = np.concatenate(
        [r["alt_t"].astype(np.float32).transpose(2, 0, 1) for r in results],
        axis=0)                                                        # [B, T, 512]
    logits = np.concatenate(
        [r["probs_t"].astype(np.float32).T[:, :, None] for r in results], axis=0)
    x = logits.astype(np.float64) + float(np.asarray(b_pe2).reshape(-1)[0])
    probs = (1.0 / (1.0 + np.exp(-x))).astype(np.float32)              # [B, T, 1]
    s = np.concatenate([r["s_t"].T for r in results], axis=0)          # [B, 128]
    final = np.ascontiguousarray(alt[:, -1, :])                        # [B, 512]
    return alt, probs, s, final


def kernel(**inputs):
    from concourse.bass_utils import run_bass_kernel_spmd

    num_steps = int(np.asarray(inputs["num_steps"]))
    nc = _get_nc(num_steps)
    in_maps = make_in_maps(inputs)
    res = run_bass_kernel_spmd(nc, in_maps, core_ids=list(range(NCORES))).results
    return assemble(res, inputs["b_pe2"])


# revision 19
# speedup vs baseline: 1.1926x; 1.1926x over previous
"""AlternateTimelineGenerator Trainium2 kernel.

Data-parallel over the batch: 16384 rows -> 8 NeuronCores x 2048 rows.
Per core, everything runs in "transposed" layout (feature on SBUF
partitions, batch rows on the free dimension) so the whole chain of
matmuls composes without on-chip transposes; only the two inputs are
transposed once at the start via TensorE. Outputs are written transposed
(fp16 for the big timeline tensor) and de-transposed/upcast on the host.

Precision plan (validated in numpy against an fp64 oracle):
  fp32: PSUM accumulation, the P = W_pe1@cur + b state, s output
  fp16: weights, sigmoid/tanh outputs, mod, cur, c0/c1
  fp8 (e4m3, DoubleRow matmuls at 0.5 cyc/row): h0, h1, rom

Per step (full 2048-row spans, matmul N=512 slices):
  gates0 = DR(W_hh0 @ h0) + W_ih0 @ s          (PE, PSUM fp32)
  i,f,g,o = ACT sigmoid/tanh (+bias, FD=2048)  -> fp16 SBUF
  c = f*c + i*g; h = o*tanh(c)                 (DVE fp16, h cast to fp8)
  gates1 = DR(W_ih1 @ h0) + DR(W_hh1 @ h1)
  rom  = relu(DR(W_om1h @ h1) + W_om1s @ s)    (PE + DVE relu -> fp8)
  mod  = tanh(DR(W_om2 @ rom) + b)             (PE + ACT -> fp16)
  P   += (0.1*W_pe1) @ mod                     (PE fp16; P = W_pe1@cur + b)
  logit= W_pe2 @ relu(P)                       (PE fp16; sigmoid on host)
  cur += 0.1 * mod                             (DVE fp16)
"""

import numpy as np
import ml_dtypes

B, FDIM, H, E = 16384, 512, 256, 128
NCORES = 8
R = B // NCORES        # rows per core
P = 128
MM = 512               # matmul moving-dim tile

_BUILD_CACHE = {}


def _build(num_steps: int, R: int = R):
    import concourse.bass as bass
    import concourse.tile as tile
    from concourse import bacc, mybir
    from concourse.masks import make_identity

    NSL = R // MM      # matmul N-slices per full span
    dt = mybir.dt
    AF = mybir.ActivationFunctionType
    ALU = mybir.AluOpType
    DR = mybir.MatmulPerfMode.DoubleRow
    f32, f16, f8 = dt.float32, dt.float16, dt.float8e4

    nc = bacc.Bacc("TRN2", target_bir_lowering=False, debug=False)

    # ---- DRAM I/O ----
    base_d = nc.dram_tensor("base", [R, FDIM], f32, kind="ExternalInput").ap()
    cf_d = nc.dram_tensor("cf", [R, FDIM], f32, kind="ExternalInput").ap()
    wt_g0h_d = nc.dram_tensor("wt_g0h", [2 * P, 4 * H], f8, kind="ExternalInput").ap()
    wt_g0s_d = nc.dram_tensor("wt_g0s", [P, 4 * H], f16, kind="ExternalInput").ap()
    wt_g1_d = nc.dram_tensor("wt_g1", [4 * P, 4 * H], f8, kind="ExternalInput").ap()
    wt_om1h_d = nc.dram_tensor("wt_om1h", [2 * P, 256], f8, kind="ExternalInput").ap()
    wt_om1s_d = nc.dram_tensor("wt_om1s", [P, 256], f16, kind="ExternalInput").ap()
    wt_om2_d = nc.dram_tensor("wt_om2", [2 * P, FDIM], f8, kind="ExternalInput").ap()
    wt_se1_d = nc.dram_tensor("wt_se1", [FDIM, 256], f16, kind="ExternalInput").ap()
    wt_se2_d = nc.dram_tensor("wt_se2", [256, E], f16, kind="ExternalInput").ap()
    wt_pe1_d = nc.dram_tensor("wt_pe1", [FDIM, E], f16, kind="ExternalInput").ap()
    wt_pe1s_d = nc.dram_tensor("wt_pe1s", [FDIM, E], f16, kind="ExternalInput").ap()
    wt_pe2_d = nc.dram_tensor("wt_pe2", [E, 1], f16, kind="ExternalInput").ap()
    bg0_d = nc.dram_tensor("bg0", [4 * H], f32, kind="ExternalInput").ap()
    bg1_d = nc.dram_tensor("bg1", [4 * H], f32, kind="ExternalInput").ap()
    bse1_d = nc.dram_tensor("b_se1", [256], f32, kind="ExternalInput").ap()
    bse2_d = nc.dram_tensor("b_se2", [E], f32, kind="ExternalInput").ap()
    bom1_d = nc.dram_tensor("b_om1", [256], f32, kind="ExternalInput").ap()
    bom2_d = nc.dram_tensor("b_om2", [FDIM], f32, kind="ExternalInput").ap()
    bpe1_d = nc.dram_tensor("b_pe1", [E], f32, kind="ExternalInput").ap()

    alt_d = nc.dram_tensor("alt_t", [num_steps, FDIM, R], f16, kind="ExternalOutput").ap()
    probs_d = nc.dram_tensor("probs_t", [num_steps, R], f16, kind="ExternalOutput").ap()
    s_d = nc.dram_tensor("s_t", [E, R], f32, kind="ExternalOutput").ap()

    with tile.TileContext(nc) as tc:
        with (
            tc.tile_pool(name="consts", bufs=1) as consts,
            tc.tile_pool(name="state", bufs=1) as state,
            tc.tile_pool(name="psum", bufs=2, space="PSUM") as psum,
        ):
            # ---- weights / biases to SBUF ----
            def w_tile(name, dram, kk, mm_, dtype):
                tl = consts.tile([P, kk, mm_], dtype, tag=name)
                nc.sync.dma_start(tl[:], dram.rearrange("(k p) m -> p k m", p=P))
                return tl

            wt_g0h = w_tile("wt_g0h", wt_g0h_d, 2, 4 * H, f8)
            wt_g0s = w_tile("wt_g0s", wt_g0s_d, 1, 4 * H, f16)
            wt_g1 = w_tile("wt_g1", wt_g1_d, 4, 4 * H, f8)
            wt_om1h = w_tile("wt_om1h", wt_om1h_d, 2, 256, f8)
            wt_om1s = w_tile("wt_om1s", wt_om1s_d, 1, 256, f16)
            wt_om2 = w_tile("wt_om2", wt_om2_d, 2, FDIM, f8)
            wt_se1 = w_tile("wt_se1", wt_se1_d, 4, 256, f16)
            wt_se2 = w_tile("wt_se2", wt_se2_d, 2, E, f16)
            wt_pe1 = w_tile("wt_pe1", wt_pe1_d, 4, E, f16)
            wt_pe1s = w_tile("wt_pe1s", wt_pe1s_d, 4, E, f16)
            wt_pe2 = consts.tile([P, 1], f16, tag="wt_pe2")
            nc.sync.dma_start(wt_pe2[:], wt_pe2_d)

            def bias_tile(name, dram, m):
                tl = consts.tile([P, m], f32, tag=name)
                nc.sync.dma_start(tl[:], dram.rearrange("(m p) -> p m", p=P))
                return tl

            bg0 = bias_tile("bg0", bg0_d, 8)
            bg1 = bias_tile("bg1", bg1_d, 8)
            bse1 = bias_tile("bse1", bse1_d, 2)
            bse2 = bias_tile("bse2", bse2_d, 1)
            bom1 = bias_tile("bom1", bom1_d, 2)
            bom2 = bias_tile("bom2", bom2_d, 4)
            bpe1 = bias_tile("bpe1", bpe1_d, 1)

            ident = consts.tile([P, P], f32, tag="ident")
            make_identity(nc, ident[:])

            # ---- persistent state ----
            h0 = state.tile([P, 2, R], f8, tag="h0")
            h1 = state.tile([P, 2, R], f8, tag="h1")
            c0 = state.tile([P, 2, R], f16, tag="c0")
            c1 = state.tile([P, 2, R], f16, tag="c1")
            cur = state.tile([P, 4, R], f16, tag="cur")
            pacc = state.tile([P, R], f32, tag="pacc")
            s = state.tile([P, R], f32, tag="s")
            sbf = state.tile([P, R], f16, tag="sbf")
            for tl in (h0, h1, c0, c1):
                nc.vector.memset(tl[:], 0.0)

            # ---- transpose inputs (cf -> cfT f16, base -> cur f16) ----
            with tc.tile_pool(name="setup", bufs=3) as setup, \
                 tc.tile_pool(name="setup1", bufs=1) as setup1:
                cfT = setup1.tile([P, 4, R], f16, tag="cfT")
                r1 = setup1.tile([P, 2, R], f16, tag="r1")
                for src, dst in ((cf_d, cfT), (base_d, cur)):
                    for rt in range(R // P):
                        tmp = setup.tile([P, FDIM], f32, tag="tr_in")
                        nc.sync.dma_start(tmp[:], src[rt * P:(rt + 1) * P, :])
                        pt = psum.tile([P, 4, P], f32, tag="ps")
                        for ft in range(4):
                            nc.tensor.transpose(
                                pt[:, ft, :], tmp[:, ft * P:(ft + 1) * P], ident[:]
                            )
                        nc.vector.tensor_copy(dst[:, :, rt * P:(rt + 1) * P], pt[:])

                # ---- scenario encoder (f16) + P0 = W_pe1 @ base^T + b ----
                for n in range(NSL):
                    sl = slice(n * MM, (n + 1) * MM)
                    for m in range(2):
                        ps = psum.tile([P, MM], f32, tag="ps")
                        for k in range(4):
                            nc.tensor.matmul(
                                ps[:], wt_se1[:, k, m * P:(m + 1) * P],
                                cfT[:, k, sl], start=(k == 0), stop=(k == 3),
                            )
                        nc.scalar.activation(
                            r1[:, m, sl], ps[:], AF.Relu, bias=bse1[:, m:m + 1])
                    ps = psum.tile([P, MM], f32, tag="ps")
                    for k in range(2):
                        nc.tensor.matmul(
                            ps[:], wt_se2[:, k, :], r1[:, k, sl],
                            start=(k == 0), stop=(k == 1),
                        )
                    nc.scalar.activation(
                        s[:, sl], ps[:], AF.Identity, bias=bse2[:, 0:1])
                    ps = psum.tile([P, MM], f32, tag="ps")
                    for k in range(4):
                        nc.tensor.matmul(
                            ps[:], wt_pe1[:, k, :], cur[:, k, sl],
                            start=(k == 0), stop=(k == 3),
                        )
                    nc.vector.tensor_scalar(
                        pacc[:, sl], ps[:], bpe1[:, 0:1], None, ALU.add)
                nc.vector.tensor_copy(sbf[:], s[:])
                nc.sync.dma_start(s_d[:, :], s[:])

            # ---- recurrence: layer-major over 2 chunks of 1024 ----
            CH = min(R, 1024)
            NCH = R // CH
            NSC = CH // MM
            with tc.tile_pool(name="loop", bufs=2) as loop, \
                 tc.tile_pool(name="psumx", bufs=2, space="PSUM") as psumx:
                for t in range(num_steps):
                    for gate_set in range(2):
                        cc = c0 if gate_set == 0 else c1
                        hh = h0 if gate_set == 0 else h1
                        bgt = bg0 if gate_set == 0 else bg1
                        sgs = {}
                        for n in range(NCH):
                            sg = loop.tile([P, 8, CH], f16, tag="sg", bufs=3)
                            sgs[n] = sg
                            for m in range(8):
                                ps = psum.tile([P, CH], f32, tag="ps")
                                for h2 in range(NSC):
                                    o = slice(n * CH + h2 * MM, n * CH + (h2 + 1) * MM)
                                    po = slice(h2 * MM, (h2 + 1) * MM)
                                    if gate_set == 1:
                                        nc.tensor.matmul(
                                            ps[:, po], wt_g1[:, 0:2, m * P:(m + 1) * P],
                                            h0[:, :, o], start=True, stop=False,
                                            perf_mode=DR)
                                        nc.tensor.matmul(
                                            ps[:, po], wt_g1[:, 2:4, m * P:(m + 1) * P],
                                            h1[:, :, o], start=False, stop=True,
                                            perf_mode=DR)
                                    else:
                                        nc.tensor.matmul(
                                            ps[:, po], wt_g0h[:, :, m * P:(m + 1) * P],
                                            h0[:, :, o], start=True, stop=False,
                                            perf_mode=DR)
                                        nc.tensor.matmul(
                                            ps[:, po], wt_g0s[:, 0, m * P:(m + 1) * P],
                                            sbf[:, o], start=False, stop=True)
                                func = AF.Tanh if m in (4, 5) else AF.Sigmoid
                                nc.scalar.activation(
                                    sg[:, m, :], ps[:], func, bias=bgt[:, m:m + 1])
                        for n in range(NCH):
                            cs = slice(n * CH, (n + 1) * CH)
                            sg = sgs[n]
                            nc.vector.tensor_tensor(
                                sg[:, 0:2, :], sg[:, 0:2, :], sg[:, 4:6, :], ALU.mult)
                            nc.vector.tensor_tensor(
                                cc[:, :, cs], sg[:, 2:4, :], cc[:, :, cs], ALU.mult)
                            nc.vector.tensor_tensor(
                                cc[:, :, cs], cc[:, :, cs], sg[:, 0:2, :], ALU.add)
                            nc.scalar.activation(sg[:, 4:6, :], cc[:, :, cs], AF.Tanh)
                            nc.vector.tensor_tensor(
                                hh[:, :, cs], sg[:, 6:8, :], sg[:, 4:6, :], ALU.mult)

                    roms = {}
                    for n in range(NCH):
                        rom = loop.tile([P, 2, CH], f8, tag="rom", bufs=2)
                        roms[n] = rom
                        for m in range(2):
                            ps = psumx.tile([P, CH], f32, tag="px")
                            for h2 in range(NSC):
                                o = slice(n * CH + h2 * MM, n * CH + (h2 + 1) * MM)
                                po = slice(h2 * MM, (h2 + 1) * MM)
                                nc.tensor.matmul(
                                    ps[:, po], wt_om1h[:, :, m * P:(m + 1) * P],
                                    h1[:, :, o], start=True, stop=False, perf_mode=DR)
                                nc.tensor.matmul(
                                    ps[:, po], wt_om1s[:, 0, m * P:(m + 1) * P],
                                    sbf[:, o], start=False, stop=True)
                            nc.vector.tensor_scalar(
                                rom[:, m, :], ps[:], bom1[:, m:m + 1], 0.0,
                                ALU.add, ALU.max)
                    mods = {}
                    for n in range(NCH):
                        rom = roms[n]
                        mod = loop.tile([P, 4, CH], f16, tag="mod", bufs=2)
                        mods[n] = mod
                        for m in range(4):
                            ps = psumx.tile([P, CH], f32, tag="px")
                            for h2 in range(NSC):
                                po = slice(h2 * MM, (h2 + 1) * MM)
                                nc.tensor.matmul(
                                    ps[:, po], wt_om2[:, :, m * P:(m + 1) * P],
                                    rom[:, :, po], start=True, stop=True, perf_mode=DR)
                            nc.scalar.activation(
                                mod[:, m, :], ps[:], AF.Tanh, bias=bom2[:, m:m + 1])
                    for n in range(NCH):
                        cs = slice(n * CH, (n + 1) * CH)
                        mod = mods[n]
                        # probability estimator: P += (0.1 W_pe1) @ mod
                        ps = psumx.tile([P, CH], f32, tag="px")
                        for h2 in range(NSC):
                            po = slice(h2 * MM, (h2 + 1) * MM)
                            for k in range(4):
                                nc.tensor.matmul(
                                    ps[:, po], wt_pe1s[:, k, :], mod[:, k, po],
                                    start=(k == 0), stop=(k == 3),
                                )
                        nc.vector.tensor_tensor(
                            pacc[:, cs], pacc[:, cs], ps[:], ALU.add)
                        rp = loop.tile([P, CH], f16, tag="rp", bufs=2)
                        nc.vector.tensor_scalar_max(rp[:], pacc[:, cs], 0.0)
                        ps2 = psumx.tile([P, CH], f32, tag="px")
                        for h2 in range(NSC):
                            po = slice(h2 * MM, (h2 + 1) * MM)
                            nc.tensor.matmul(
                                ps2[0:1, po], wt_pe2[:, 0:1], rp[:, po],
                                start=True, stop=True,
                            )
                        pb = loop.tile([1, CH], f16, tag="pb", bufs=2)
                        nc.vector.tensor_copy(pb[0:1, :], ps2[0:1, :])
                        nc.sync.dma_start(probs_d[t:t + 1, cs], pb[0:1, :])
                        # cur += 0.1*mod ; stream out
                        nc.vector.scalar_tensor_tensor(
                            cur[:, :, cs], mod[:], 0.1, cur[:, :, cs],
                            ALU.mult, ALU.add)
                        nc.sync.dma_start(
                            alt_d[t].rearrange("(fc p) r -> p fc r", p=P)[:, :, cs],
                            cur[:, :, cs],
                        )

    nc.compile()
    return nc


def _get_nc(num_steps: int):
    if num_steps not in _BUILD_CACHE:
        _BUILD_CACHE[num_steps] = _build(num_steps)
    return _BUILD_CACHE[num_steps]


def make_in_maps(inputs, R_=None):
    """Shard inputs + preprocess params into per-core input maps."""
    def npf(x):
        return np.asarray(x, np.float32)

    f16 = np.float16
    f8 = ml_dtypes.float8_e4m3fn
    params = {
        "wt_g0h": np.ascontiguousarray(npf(inputs["w_hh0"]).T).astype(f8),
        "wt_g0s": np.ascontiguousarray(npf(inputs["w_ih0"]).T).astype(f16),
        "wt_g1": np.ascontiguousarray(
            np.concatenate([npf(inputs["w_ih1"]).T, npf(inputs["w_hh1"]).T], 0)
        ).astype(f8),
        "wt_om1h": np.ascontiguousarray(npf(inputs["w_om1"]).T[:256]).astype(f8),
        "wt_om1s": np.ascontiguousarray(npf(inputs["w_om1"]).T[256:]).astype(f16),
        "wt_om2": np.ascontiguousarray(npf(inputs["w_om2"]).T).astype(f8),
        "wt_se1": np.ascontiguousarray(npf(inputs["w_se1"]).T).astype(f16),
        "wt_se2": np.ascontiguousarray(npf(inputs["w_se2"]).T).astype(f16),
        "wt_pe1": np.ascontiguousarray(npf(inputs["w_pe1"]).T).astype(f16),
        "wt_pe1s": np.ascontiguousarray(0.1 * npf(inputs["w_pe1"]).T).astype(f16),
        "wt_pe2": np.ascontiguousarray(npf(inputs["w_pe2"]).T).astype(f16),
        "bg0": npf(inputs["b_ih0"]) + npf(inputs["b_hh0"]),
        "bg1": npf(inputs["b_ih1"]) + npf(inputs["b_hh1"]),
        "b_se1": npf(inputs["b_se1"]),
        "b_se2": npf(inputs["b_se2"]),
        "b_om1": npf(inputs["b_om1"]),
        "b_om2": npf(inputs["b_om2"]),
        "b_pe1": npf(inputs["b_pe1"]),
    }
    rr = R_ or R
    base = npf(inputs["base_timeline"])
    cf = npf(inputs["counterfactual_scenario"])
    ncores = base.shape[0] // rr
    in_maps = []
    for i in range(ncores):
        m = dict(params)
        m["base"] = np.ascontiguousarray(base[i * rr:(i + 1) * rr])
        m["cf"] = np.ascontiguousarray(cf[i * rr:(i + 1) * rr])
        in_maps.append(m)
    return in_maps


def assemble(results, b_pe2=0.0):
    """Gather per-core transposed outputs into full reference-shaped arrays."""
    alt = np.concatenate(
        [r["alt_t"].astype(np.float32).transpose(2, 0, 1) for r in results],
        axis=0)                                                        # [B, T, 512]
    logits = np.concatenate(
        [r["probs_t"].astype(np.float32).T[:, :, None] for r in results], axis=0)
    x = logits.astype(np.float64) + float(np.asarray(b_pe2).reshape(-1)[0])
    probs = (1.0 / (1.0 + np.exp(-x))).astype(np.float32)              # [B, T, 1]
    s = np.concatenate([r["s_t"].T for r in results], axis=0)          # [B, 128]
    final = np.ascontiguousarray(alt[:, -1, :])                        # [B, 512]
    return alt, probs, s, final


def kernel(**inputs):
    from concourse.bass_utils import run_bass_kernel_spmd

    num_steps = int(np.asarray(inputs["num_steps"]))
    nc = _get_nc(num_steps)
    in_maps = make_in_maps(inputs)
    res = run_bass_kernel_spmd(nc, in_maps, core_ids=list(range(NCORES))).results
    return assemble(res, inputs["b_pe2"])


# revision 20
# speedup vs baseline: 1.2920x; 1.0833x over previous
"""AlternateTimelineGenerator Trainium2 kernel.

Data-parallel over the batch: 16384 rows -> 8 NeuronCores x 2048 rows.
Per core, everything runs in "transposed" layout (feature on SBUF
partitions, batch rows on the free dimension) so the whole chain of
matmuls composes without on-chip transposes; only the two inputs are
transposed once at the start via TensorE. Outputs are written transposed
(fp16 for the big timeline tensor) and de-transposed/upcast on the host.

Precision plan (validated in numpy against an fp64 oracle):
  fp32: PSUM accumulation, the P = W_pe1@cur + b state, s output
  fp16: weights, sigmoid/tanh outputs, mod, cur, c0/c1
  fp8 (e4m3, DoubleRow matmuls at 0.5 cyc/row): h0, h1, rom

Per step (full 2048-row spans, matmul N=512 slices):
  gates0 = DR(W_hh0 @ h0) + W_ih0 @ s          (PE, PSUM fp32)
  i,f,g,o = ACT sigmoid/tanh (+bias, FD=2048)  -> fp16 SBUF
  c = f*c + i*g; h = o*tanh(c)                 (DVE fp16, h cast to fp8)
  gates1 = DR(W_ih1 @ h0) + DR(W_hh1 @ h1)
  rom  = relu(DR(W_om1h @ h1) + W_om1s @ s)    (PE + DVE relu -> fp8)
  mod  = tanh(DR(W_om2 @ rom) + b)             (PE + ACT -> fp16)
  P   += (0.1*W_pe1) @ mod                     (PE fp16; P = W_pe1@cur + b)
  logit= W_pe2 @ relu(P)                       (PE fp16; sigmoid on host)
  cur += 0.1 * mod                             (DVE fp16)
"""

import numpy as np
import ml_dtypes

B, FDIM, H, E = 16384, 512, 256, 128
NCORES = 8
R = B // NCORES        # rows per core
P = 128
MM = 512               # matmul moving-dim tile

_BUILD_CACHE = {}


def _build(num_steps: int, R: int = R):
    import concourse.bass as bass
    import concourse.tile as tile
    from concourse import bacc, mybir
    from concourse.masks import make_identity

    NSL = R // MM      # matmul N-slices per full span
    dt = mybir.dt
    AF = mybir.ActivationFunctionType
    ALU = mybir.AluOpType
    DR = mybir.MatmulPerfMode.DoubleRow
    f32, f16, f8 = dt.float32, dt.float16, dt.float8e4

    nc = bacc.Bacc("TRN2", target_bir_lowering=False, debug=False)

    # ---- DRAM I/O ----
    base_d = nc.dram_tensor("base", [R, FDIM], f32, kind="ExternalInput").ap()
    cf_d = nc.dram_tensor("cf", [R, FDIM], f32, kind="ExternalInput").ap()
    wt_g0h_d = nc.dram_tensor("wt_g0h", [2 * P, 4 * H], f8, kind="ExternalInput").ap()
    wt_g0s_d = nc.dram_tensor("wt_g0s", [P, 4 * H], f16, kind="ExternalInput").ap()
    wt_g1_d = nc.dram_tensor("wt_g1", [4 * P, 4 * H], f8, kind="ExternalInput").ap()
    wt_om1h_d = nc.dram_tensor("wt_om1h", [2 * P, 256], f8, kind="ExternalInput").ap()
    wt_om1s_d = nc.dram_tensor("wt_om1s", [P, 256], f16, kind="ExternalInput").ap()
    wt_om2_d = nc.dram_tensor("wt_om2", [2 * P, FDIM], f8, kind="ExternalInput").ap()
    wt_se1_d = nc.dram_tensor("wt_se1", [FDIM, 256], f16, kind="ExternalInput").ap()
    wt_se2_d = nc.dram_tensor("wt_se2", [256, E], f16, kind="ExternalInput").ap()
    wt_pe1_d = nc.dram_tensor("wt_pe1", [FDIM, E], f16, kind="ExternalInput").ap()
    wt_pe2_d = nc.dram_tensor("wt_pe2", [E, 1], f16, kind="ExternalInput").ap()
    bg0_d = nc.dram_tensor("bg0", [4 * H], f32, kind="ExternalInput").ap()
    bg1_d = nc.dram_tensor("bg1", [4 * H], f32, kind="ExternalInput").ap()
    bse1_d = nc.dram_tensor("b_se1", [256], f32, kind="ExternalInput").ap()
    bse2_d = nc.dram_tensor("b_se2", [E], f32, kind="ExternalInput").ap()
    bom1_d = nc.dram_tensor("b_om1", [256], f32, kind="ExternalInput").ap()
    bom2_d = nc.dram_tensor("b_om2", [FDIM], f32, kind="ExternalInput").ap()
    bpe1_d = nc.dram_tensor("b_pe1", [E], f32, kind="ExternalInput").ap()

    alt_d = nc.dram_tensor("alt_t", [num_steps, FDIM, R], f16, kind="ExternalOutput").ap()
    probs_d = nc.dram_tensor("probs_t", [num_steps, R], f16, kind="ExternalOutput").ap()
    s_d = nc.dram_tensor("s_t", [E, R], f32, kind="ExternalOutput").ap()

    with tile.TileContext(nc) as tc:
        with (
            tc.tile_pool(name="consts", bufs=1) as consts,
            tc.tile_pool(name="state", bufs=1) as state,
            tc.tile_pool(name="psum", bufs=2, space="PSUM") as psum,
        ):
            # ---- weights / biases to SBUF ----
            def w_tile(name, dram, kk, mm_, dtype):
                tl = consts.tile([P, kk, mm_], dtype, tag=name)
                nc.sync.dma_start(tl[:], dram.rearrange("(k p) m -> p k m", p=P))
                return tl

            wt_g0h = w_tile("wt_g0h", wt_g0h_d, 2, 4 * H, f8)
            wt_g0s = w_tile("wt_g0s", wt_g0s_d, 1, 4 * H, f16)
            wt_g1 = w_tile("wt_g1", wt_g1_d, 4, 4 * H, f8)
            wt_om1h = w_tile("wt_om1h", wt_om1h_d, 2, 256, f8)
            wt_om1s = w_tile("wt_om1s", wt_om1s_d, 1, 256, f16)
            wt_om2 = w_tile("wt_om2", wt_om2_d, 2, FDIM, f8)
            wt_se1 = w_tile("wt_se1", wt_se1_d, 4, 256, f16)
            wt_se2 = w_tile("wt_se2", wt_se2_d, 2, E, f16)
            wt_pe1 = w_tile("wt_pe1", wt_pe1_d, 4, E, f16)
            wt_pe2 = consts.tile([P, 1], f16, tag="wt_pe2")
            nc.sync.dma_start(wt_pe2[:], wt_pe2_d)

            def bias_tile(name, dram, m):
                tl = consts.tile([P, m], f32, tag=name)
                nc.sync.dma_start(tl[:], dram.rearrange("(m p) -> p m", p=P))
                return tl

            bg0 = bias_tile("bg0", bg0_d, 8)
            bg1 = bias_tile("bg1", bg1_d, 8)
            bse1 = bias_tile("bse1", bse1_d, 2)
            bse2 = bias_tile("bse2", bse2_d, 1)
            bom1 = bias_tile("bom1", bom1_d, 2)
            bom2 = bias_tile("bom2", bom2_d, 4)
            bpe1 = bias_tile("bpe1", bpe1_d, 1)

            ident = consts.tile([P, P], f32, tag="ident")
            make_identity(nc, ident[:])

            # ---- persistent state ----
            h0 = state.tile([P, 2, R], f8, tag="h0")
            h1 = state.tile([P, 2, R], f8, tag="h1")
            c0 = state.tile([P, 2, R], f16, tag="c0")
            c1 = state.tile([P, 2, R], f16, tag="c1")
            cur = state.tile([P, 4, R], f16, tag="cur")
            pacc = state.tile([P, R], f32, tag="pacc")
            s = state.tile([P, R], f32, tag="s")
            sbf = state.tile([P, R], f16, tag="sbf")
            for tl in (h0, h1, c0, c1):
                nc.vector.memset(tl[:], 0.0)

            # ---- transpose inputs (cf -> cfT f16, base -> cur f16) ----
            with tc.tile_pool(name="setup", bufs=3) as setup, \
                 tc.tile_pool(name="setup1", bufs=1) as setup1:
                cfT = setup1.tile([P, 4, R], f16, tag="cfT")
                r1 = setup1.tile([P, 2, R], f16, tag="r1")
                for src, dst in ((cf_d, cfT), (base_d, cur)):
                    for rt in range(R // P):
                        tmp = setup.tile([P, FDIM], f32, tag="tr_in")
                        nc.sync.dma_start(tmp[:], src[rt * P:(rt + 1) * P, :])
                        pt = psum.tile([P, 4, P], f32, tag="ps")
                        for ft in range(4):
                            nc.tensor.transpose(
                                pt[:, ft, :], tmp[:, ft * P:(ft + 1) * P], ident[:]
                            )
                        nc.vector.tensor_copy(dst[:, :, rt * P:(rt + 1) * P], pt[:])

                # ---- scenario encoder (f16) + P0 = W_pe1 @ base^T + b ----
                for n in range(NSL):
                    sl = slice(n * MM, (n + 1) * MM)
                    for m in range(2):
                        ps = psum.tile([P, MM], f32, tag="ps")
                        for k in range(4):
                            nc.tensor.matmul(
                                ps[:], wt_se1[:, k, m * P:(m + 1) * P],
                                cfT[:, k, sl], start=(k == 0), stop=(k == 3),
                            )
                        nc.scalar.activation(
                            r1[:, m, sl], ps[:], AF.Relu, bias=bse1[:, m:m + 1])
                    ps = psum.tile([P, MM], f32, tag="ps")
                    for k in range(2):
                        nc.tensor.matmul(
                            ps[:], wt_se2[:, k, :], r1[:, k, sl],
                            start=(k == 0), stop=(k == 1),
                        )
                    nc.scalar.activation(
                        s[:, sl], ps[:], AF.Identity, bias=bse2[:, 0:1])
                    ps = psum.tile([P, MM], f32, tag="ps")
                    for k in range(4):
                        nc.tensor.matmul(
                            ps[:], wt_pe1[:, k, :], cur[:, k, sl],
                            start=(k == 0), stop=(k == 3),
                        )
                    nc.vector.tensor_scalar(
                        pacc[:, sl], ps[:], bpe1[:, 0:1], None, ALU.add)
                nc.vector.tensor_copy(sbf[:], s[:])
                nc.sync.dma_start(s_d[:, :], s[:])

            # ---- recurrence: layer-major over 2 chunks, software-pipelined ----
            CH = min(R, 1024)
            NCH = R // CH
            NSC = CH // MM
            with tc.tile_pool(name="loop", bufs=2) as loop, \
                 tc.tile_pool(name="psumx", bufs=2, space="PSUM") as psumx:

                def gates_phase(gate_set):
                    cc = c0 if gate_set == 0 else c1
                    hh = h0 if gate_set == 0 else h1
                    bgt = bg0 if gate_set == 0 else bg1
                    sgs = {}
                    for n in range(NCH):
                        sg = loop.tile([P, 8, CH], f16, tag="sg", bufs=4)
                        sgs[n] = sg
                        for m in range(8):
                            ps = psum.tile([P, CH], f32, tag="ps")
                            for h2 in range(NSC):
                                o = slice(n * CH + h2 * MM, n * CH + (h2 + 1) * MM)
                                po = slice(h2 * MM, (h2 + 1) * MM)
                                if gate_set == 1:
                                    nc.tensor.matmul(
                                        ps[:, po], wt_g1[:, 0:2, m * P:(m + 1) * P],
                                        h0[:, :, o], start=True, stop=False,
                                        perf_mode=DR)
                                    nc.tensor.matmul(
                                        ps[:, po], wt_g1[:, 2:4, m * P:(m + 1) * P],
                                        h1[:, :, o], start=False, stop=True,
                                        perf_mode=DR)
                                else:
                                    nc.tensor.matmul(
                                        ps[:, po], wt_g0h[:, :, m * P:(m + 1) * P],
                                        h0[:, :, o], start=True, stop=False,
                                        perf_mode=DR)
                                    nc.tensor.matmul(
                                        ps[:, po], wt_g0s[:, 0, m * P:(m + 1) * P],
                                        sbf[:, o], start=False, stop=True)
                            func = AF.Tanh if m in (4, 5) else AF.Sigmoid
                            nc.scalar.activation(
                                sg[:, m, :], ps[:], func, bias=bgt[:, m:m + 1])
                    return sgs

                def cell_phase(gate_set, sgs):
                    cc = c0 if gate_set == 0 else c1
                    hh = h0 if gate_set == 0 else h1
                    for n in range(NCH):
                        cs = slice(n * CH, (n + 1) * CH)
                        sg = sgs[n]
                        nc.vector.tensor_tensor(
                            sg[:, 0:2, :], sg[:, 0:2, :], sg[:, 4:6, :], ALU.mult)
                        nc.vector.tensor_tensor(
                            cc[:, :, cs], sg[:, 2:4, :], cc[:, :, cs], ALU.mult)
                        nc.vector.tensor_tensor(
                            cc[:, :, cs], cc[:, :, cs], sg[:, 0:2, :], ALU.add)
                        nc.scalar.activation(sg[:, 4:6, :], cc[:, :, cs], AF.Tanh)
                        nc.vector.tensor_tensor(
                            hh[:, :, cs], sg[:, 6:8, :], sg[:, 4:6, :], ALU.mult)

                def om_pe_phase(t):
                    roms = {}
                    for n in range(NCH):
                        rom = loop.tile([P, 2, CH], f8, tag="rom", bufs=2)
                        roms[n] = rom
                        for m in range(2):
                            ps = psumx.tile([P, CH], f32, tag="px")
                            for h2 in range(NSC):
                                o = slice(n * CH + h2 * MM, n * CH + (h2 + 1) * MM)
                                po = slice(h2 * MM, (h2 + 1) * MM)
                                nc.tensor.matmul(
                                    ps[:, po], wt_om1h[:, :, m * P:(m + 1) * P],
                                    h1[:, :, o], start=True, stop=False, perf_mode=DR)
                                nc.tensor.matmul(
                                    ps[:, po], wt_om1s[:, 0, m * P:(m + 1) * P],
                                    sbf[:, o], start=False, stop=True)
                            nc.vector.tensor_scalar(
                                rom[:, m, :], ps[:], bom1[:, m:m + 1], 0.0,
                                ALU.add, ALU.max)
                    for n in range(NCH):
                        cs = slice(n * CH, (n + 1) * CH)
                        rom = roms[n]
                        mod = loop.tile([P, 4, CH], f16, tag="mod", bufs=2)
                        for m in range(4):
                            ps = psumx.tile([P, CH], f32, tag="px")
                            for h2 in range(NSC):
                                po = slice(h2 * MM, (h2 + 1) * MM)
                                nc.tensor.matmul(
                                    ps[:, po], wt_om2[:, :, m * P:(m + 1) * P],
                                    rom[:, :, po], start=True, stop=True, perf_mode=DR)
                            nc.scalar.activation(
                                mod[:, m, :], ps[:], AF.Tanh, bias=bom2[:, m:m + 1])
                        # mod *= 0.1 once; consumers use unscaled weights
                        nc.vector.tensor_scalar_mul(mod[:], mod[:], 0.1)
                        # probability estimator: P += W_pe1 @ (0.1 mod)
                        ps = psumx.tile([P, CH], f32, tag="px")
                        for h2 in range(NSC):
                            po = slice(h2 * MM, (h2 + 1) * MM)
                            for k in range(4):
                                nc.tensor.matmul(
                                    ps[:, po], wt_pe1[:, k, :], mod[:, k, po],
                                    start=(k == 0), stop=(k == 3),
                                )
                        nc.vector.tensor_tensor(
                            pacc[:, cs], pacc[:, cs], ps[:], ALU.add)
                        rp = loop.tile([P, CH], f16, tag="rp", bufs=2)
                        nc.vector.tensor_scalar_max(rp[:], pacc[:, cs], 0.0)
                        ps2 = psumx.tile([P, CH], f32, tag="px")
                        for h2 in range(NSC):
                            po = slice(h2 * MM, (h2 + 1) * MM)
                            nc.tensor.matmul(
                                ps2[0:1, po], wt_pe2[:, 0:1], rp[:, po],
                                start=True, stop=True,
                            )
                        pb = loop.tile([1, CH], f16, tag="pb", bufs=2)
                        nc.vector.tensor_copy(pb[0:1, :], ps2[0:1, :])
                        nc.sync.dma_start(probs_d[t:t + 1, cs], pb[0:1, :])
                        # cur += 0.1*mod ; stream out
                        nc.vector.tensor_tensor(
                            cur[:, :, cs], cur[:, :, cs], mod[:], ALU.add)
                        nc.sync.dma_start(
                            alt_d[t].rearrange("(fc p) r -> p fc r", p=P)[:, :, cs],
                            cur[:, :, cs],
                        )

                sg0 = gates_phase(0)
                for t in range(num_steps):
                    cell_phase(0, sg0)
                    sg1 = gates_phase(1)
                    cell_phase(1, sg1)
                    if t + 1 < num_steps:
                        sg0 = gates_phase(0)
                    om_pe_phase(t)

    nc.compile()
    return nc


def _get_nc(num_steps: int):
    if num_steps not in _BUILD_CACHE:
        _BUILD_CACHE[num_steps] = _build(num_steps)
    return _BUILD_CACHE[num_steps]


def make_in_maps(inputs, R_=None):
    """Shard inputs + preprocess params into per-core input maps."""
    def npf(x):
        return np.asarray(x, np.float32)

    f16 = np.float16
    f8 = ml_dtypes.float8_e4m3fn
    params = {
        "wt_g0h": np.ascontiguousarray(npf(inputs["w_hh0"]).T).astype(f8),
        "wt_g0s": np.ascontiguousarray(npf(inputs["w_ih0"]).T).astype(f16),
        "wt_g1": np.ascontiguousarray(
            np.concatenate([npf(inputs["w_ih1"]).T, npf(inputs["w_hh1"]).T], 0)
        ).astype(f8),
        "wt_om1h": np.ascontiguousarray(npf(inputs["w_om1"]).T[:256]).astype(f8),
        "wt_om1s": np.ascontiguousarray(npf(inputs["w_om1"]).T[256:]).astype(f16),
        "wt_om2": np.ascontiguousarray(npf(inputs["w_om2"]).T).astype(f8),
        "wt_se1": np.ascontiguousarray(npf(inputs["w_se1"]).T).astype(f16),
        "wt_se2": np.ascontiguousarray(npf(inputs["w_se2"]).T).astype(f16),
        "wt_pe1": np.ascontiguousarray(npf(inputs["w_pe1"]).T).astype(f16),
        "wt_pe2": np.ascontiguousarray(npf(inputs["w_pe2"]).T).astype(f16),
        "bg0": npf(inputs["b_ih0"]) + npf(inputs["b_hh0"]),
        "bg1": npf(inputs["b_ih1"]) + npf(inputs["b_hh1"]),
        "b_se1": npf(inputs["b_se1"]),
        "b_se2": npf(inputs["b_se2"]),
        "b_om1": npf(inputs["b_om1"]),
        "b_om2": npf(inputs["b_om2"]),
        "b_pe1": npf(inputs["b_pe1"]),
    }
    rr = R_ or R
    base = npf(inputs["base_timeline"])
    cf = npf(inputs["counterfactual_scenario"])
    ncores = base.shape[0] // rr
    in_maps = []
    for i in range(ncores):
        m = dict(params)
        m["base"] = np.ascontiguousarray(base[i * rr:(i + 1) * rr])
        m["cf"] = np.ascontiguousarray(cf[i * rr:(i + 1) * rr])
        in_maps.append(m)
    return in_maps


def assemble(results, b_pe2=0.0):
    """Gather per-core transposed outputs into full reference-shaped arrays."""
    alt = np.concatenate(
        [r["alt_t"].astype(np.float32).transpose(2, 0, 1) for r in results],
        axis=0)                                                        # [B, T, 512]
    logits = np.concatenate(
        [r["probs_t"].astype(np.float32).T[:, :, None] for r in results], axis=0)
    x = logits.astype(np.float64) + float(np.asarray(b_pe2).reshape(-1)[0])
    probs = (1.0 / (1.0 + np.exp(-x))).astype(np.float32)              # [B, T, 1]
    s = np.concatenate([r["s_t"].T for r in results], axis=0)          # [B, 128]
    final = np.ascontiguousarray(alt[:, -1, :])                        # [B, 512]
    return alt, probs, s, final


def kernel(**inputs):
    from concourse.bass_utils import run_bass_kernel_spmd

    num_steps = int(np.asarray(inputs["num_steps"]))
    nc = _get_nc(num_steps)
    in_maps = make_in_maps(inputs)
    res = run_bass_kernel_spmd(nc, in_maps, core_ids=list(range(NCORES))).results
    return assemble(res, inputs["b_pe2"])
